# revision 2
# baseline (speedup 1.0000x reference)
"""ChebConvNet (K=3, 5 conv layers + pool + MLP) on 8 TRN2 NeuronCores — v4.

v2 base (bf16 datapath; host scatter one-hots; serpentine balance; lo/hi
overlap split; pipelined dma_gather + PE scatter matmuls) plus:
 - last conv layer + pool folded into precomputed pooled rows B/BS/BS2
   (g = (B h)(W0-W2) + (BS h)W1 + (BS2 h)(2W2) + cnt*b2): removes 2 spmv
   passes and 2 AllGathers.
 - split AllGather: table rows are relabeled so each rank's local rows
   [0,NPIECE0) form a contiguous leading block of the table; the AG for
   that block fires mid-pass (hidden under remaining gather descriptor
   generation) and only the small tail AG is exposed.
 - incremental table writes: transpose/scale/DMA of table chunks runs
   every 2 windows inside the pass instead of as a post-pass phase.
 - pool phase folded into the last pass the same way (per-chunk
   transpose+pool matmul as H windows complete).
"""
import numpy as np
import concourse.bacc as bacc
import concourse.bass as bass
import concourse.mybir as mybir
import concourse.tile as tile
from concourse.masks import make_identity

F32 = mybir.dt.float32
BF16 = mybir.dt.bfloat16
I16 = mybir.dt.int16
AF = mybir.ActivationFunctionType
ALU = mybir.AluOpType

P = 128
CH = 128
NUM_HIDDENS = 3
NUM_GRAPHS = 64
D_OUT_HID = 32
D_OUT = 16
NCORES = 8

NL = 6272            # padded nodes per core (98 subwindows of 64)
NTOT = NL * NCORES   # 50176 padded total
SW = 64              # subwindow width (dst cols per scatter matmul)
SPW = 7              # subwindows per PSUM window
WW = SW * SPW        # 448 dst per window
NSUB = NL // SW      # 98
NW = NSUB // SPW     # 14 windows
NCHN = NL // P       # 49 node-major chunks per core
BASE_HI = NTOT - 32768  # 17408; lo rows [0,32768), hi rows [17408, NTOT)

N_CONV = 1 + NUM_HIDDENS      # device conv layers (last conv folded into pool)
# split-AG: table rows are rank-major within two pieces; piece 0 holds each
# rank's local rows [0, NPIECE0) (= windows 0..9), piece 1 the rest.
NPIECE0 = NL                  # single-piece AG (split-AG regressed: Local
NPIECE1 = 0                   # collective output runs at ~half Shared BW)
T0TOT = NPIECE0 * NCORES
MID_W = None                  # no mid-pass AG

import os as _os_mod
SP = bool(int(_os_mod.environ.get("CHEB_SP", "0")))  # dma_gather single_packet


def cdiv(a, b):
    return (a + b - 1) // b


class Prep:
    """Host-side: permutation, edge bucketing, idx streams, m matrices."""

    def __init__(self, n_nodes, edge_index):
        N = self.N = n_nodes
        src = np.asarray(edge_index[0], dtype=np.int64)
        dst = np.asarray(edge_index[1], dtype=np.int64)
        keep = src != dst
        src, dst = src[keep], dst[keep]
        self.src, self.dst = src, dst
        deg = np.bincount(src, minlength=N).astype(np.float64)
        self.dinv = np.where(deg > 0, 1.0 / np.sqrt(np.maximum(deg, 1.0)),
                             0.0).astype(np.float32)

        # ---- serpentine in-degree balancing over 784 buckets of 64 slots
        in_deg = np.zeros(NTOT, np.int64)
        in_deg[:N] = np.bincount(dst, minlength=N)
        order = np.argsort(-in_deg, kind="stable")  # pads (deg 0) at end
        NB = NCORES * NSUB  # 784 buckets
        pos = np.empty(NTOT, np.int64)
        for r in range(SW):
            sl = order[r * NB:(r + 1) * NB]
            b = np.arange(NB) if r % 2 == 0 else np.arange(NB)[::-1]
            pos[sl] = (b % NCORES) * NL + (b // NCORES) * SW + r
        self.pos = pos  # old id -> new id

        # table-row relabeling: position (core k, local row r) -> table row
        # k*NPIECE0 + r for r < NPIECE0, else T0TOT + k*NPIECE1 + (r-NPIECE0).
        # Makes each AG piece a contiguous table block while keeping the
        # dst-side (window/subwindow) layout untouched.
        pidx = np.arange(NTOT)
        rr, kk = pidx % NL, pidx // NL
        self.trow = np.where(rr < NPIECE0, kk * NPIECE0 + rr,
                             T0TOT + kk * NPIECE1 + (rr - NPIECE0))

        ps = self.trow[pos[src]]
        pd = pos[dst]
        core = pd // NL
        rem = pd % NL
        sub = rem // SW
        dcol = rem % SW
        wdst = -self.dinv[dst]  # m value

        # ---- half split: fixed lo (<BASE_HI), fixed hi (>=32768),
        # flexible in [BASE_HI, 32768) balanced per (core, sub) bucket
        half = np.where(ps < BASE_HI, 0, np.where(ps >= 32768, 1, -1))
        bucket_id = core * NSUB + sub
        bord = np.argsort(bucket_id, kind="stable")
        bounds = np.searchsorted(bucket_id[bord], np.arange(NB + 1))
        for b in range(NB):
            seg = bord[bounds[b]:bounds[b + 1]]
            if len(seg) == 0:
                continue
            hm = half[seg]
            nlo = int((hm == 0).sum())
            flex = seg[hm == -1]
            tot = len(seg)
            want_lo = max(0, min(len(flex), (tot + 1) // 2 - nlo))
            half[flex[:want_lo]] = 0
            half[flex[want_lo:]] = 1

        w = sub // SPW
        s_in_w = sub % SPW
        cnt = np.zeros((NCORES, NW, SPW, 2), np.int64)
        np.add.at(cnt, (core, w, s_in_w, half), 1)
        K = cdiv(cnt.max(axis=0), 128)  # [NW, SPW, 2]
        K[:, :, 0] = np.maximum(K[:, :, 0], 1)
        self.K = K
        self.S = K.sum(axis=1) * 128 // 16  # [NW, 2] idx cols per call
        self.Stot = self.S.sum(axis=0)      # [2]
        self.nslot = int(K.sum())
        self.Kmax = int(K.sum(axis=1).max())

        # slot order: w asc, s asc, h asc, c asc (must match device loops)
        slot_of = np.zeros((NW, SPW, 2), np.int64)
        t = 0
        for wi in range(NW):
            for si in range(SPW):
                for hi in range(2):
                    slot_of[wi, si, hi] = t
                    t += int(K[wi, si, hi])

        self.idx_in = []
        self.m_in = []
        for k in range(NCORES):
            msk = core == k
            kh, kw, kss, kd, kps, kwd = (half[msk], w[msk], s_in_w[msk],
                                         dcol[msk], ps[msk], wdst[msk])
            idx_h = [np.zeros((P, max(int(self.Stot[h]), 1)), np.int16)
                     for h in range(2)]
            m_all = np.zeros((P, self.nslot * SW), np.float32)
            soff = [0, 0]
            for wi in range(NW):
                for h in range(2):
                    stream = []
                    for si in range(SPW):
                        emsk = (kw == wi) & (kss == si) & (kh == h)
                        e_d = kd[emsk]
                        e_ps = kps[emsk]
                        e_wd = kwd[emsk]
                        kk = int(K[wi, si, h])
                        npad = kk * 128 - len(e_ps)
                        assert npad >= 0, (wi, si, h, len(e_ps))
                        base = 0 if h == 0 else BASE_HI
                        stream.append(np.concatenate(
                            [e_ps - base, np.zeros(npad, np.int64)]))
                        sl0 = slot_of[wi, si, h]
                        i = np.arange(len(e_d))
                        np.add.at(m_all, (i % 128,
                                          (sl0 + i // 128) * SW + e_d), e_wd)
                    st = np.concatenate(stream)
                    S_w = int(self.S[wi, h])
                    if S_w:
                        wrapped = st.reshape(S_w, 16).T.astype(np.int16)
                        idx_h[h][:, soff[h]:soff[h] + S_w] = np.tile(
                            wrapped, (8, 1))
                    soff[h] += S_w
            self.idx_in.append(idx_h)
            self.m_in.append(m_all)


def build_kernel(prep: Prep):
    NW_MATS = 3 + NUM_HIDDENS * 3 + 3  # 12 conv + 3 pool-combo
    K, S = prep.K, prep.S
    nc = bacc.Bacc("TRN2", target_bir_lowering=False, debug=False,
                   num_devices=NCORES, num_swdge_queues=4)
    rg = [list(range(NCORES))]

    tbl0_d = nc.dram_tensor("tbl0", [NTOT, CH], BF16, kind="ExternalInput")
    h0_d = nc.dram_tensor("h0", [P, NL], BF16, kind="ExternalInput")
    idx_lo_d = nc.dram_tensor("idx_lo", [P, max(prep.Stot[0], 1)], I16,
                              kind="ExternalInput")
    idx_hi_d = nc.dram_tensor("idx_hi", [P, max(prep.Stot[1], 1)], I16,
                              kind="ExternalInput")
    m_d = nc.dram_tensor("m_all", [P, prep.nslot * SW], BF16,
                         kind="ExternalInput")
    dnm_d = nc.dram_tensor("dinv_nm", [P, NCHN], F32, kind="ExternalInput")
    d2nm_d = nc.dram_tensor("dinv2_nm", [P, NCHN], F32, kind="ExternalInput")
    pool_d = nc.dram_tensor("poolmat", [P, NCHN * 3 * NUM_GRAPHS], BF16,
                            kind="ExternalInput")
    w_d = nc.dram_tensor("Wcat", [P, NW_MATS * CH], BF16,
                         kind="ExternalInput")
    b_d = nc.dram_tensor("bcat", [P, 1 + NUM_HIDDENS], F32,
                         kind="ExternalInput")
    bias_d = nc.dram_tensor("bias_mat", [P, NUM_GRAPHS], F32,
                            kind="ExternalInput")
    p1_d = nc.dram_tensor("P1", [P, D_OUT_HID], F32, kind="ExternalInput")
    p2_d = nc.dram_tensor("P2", [D_OUT_HID, D_OUT], F32, kind="ExternalInput")
    pb1_d = nc.dram_tensor("pb1", [D_OUT_HID, 1], F32, kind="ExternalInput")
    pb2_d = nc.dram_tensor("pb2", [D_OUT, 1], F32, kind="ExternalInput")
    out_d = nc.dram_tensor("out", [D_OUT, NUM_GRAPHS], F32,
                           kind="ExternalOutput")

    with tile.TileContext(nc) as tc:
        with (
            tc.tile_pool(name="static", bufs=1) as st,
            tc.tile_pool(name="feat", bufs=1) as feat,
            tc.tile_pool(name="vlo", bufs=4) as vlo_p,
            tc.tile_pool(name="vhi", bufs=4) as vhi_p,
            tc.tile_pool(name="nm", bufs=2) as nm_p,
            tc.tile_pool(name="psu", bufs=2, space="PSUM") as psu,
            tc.tile_pool(name="psw", bufs=2, space="PSUM") as psw,
            tc.tile_pool(name="pst", bufs=2, space="PSUM") as pst,
            tc.tile_pool(name="psg", bufs=1, space="PSUM") as psg,
            tc.tile_pool(name="dram", bufs=1, space="DRAM") as dram,
        ):
            idx_lo = st.tile([P, max(prep.Stot[0], 1)], I16)
            nc.sync.dma_start(out=idx_lo[:], in_=idx_lo_d.ap())
            idx_hi = st.tile([P, max(prep.Stot[1], 1)], I16)
            nc.sync.dma_start(out=idx_hi[:], in_=idx_hi_d.ap())
            ident = st.tile([P, P], BF16)
            make_identity(nc, ident[:])
            m_all = st.tile([P, prep.nslot * SW], BF16)
            nc.sync.dma_start(out=m_all[:], in_=m_d.ap())
            dnm = st.tile([P, NCHN], F32)
            nc.sync.dma_start(out=dnm[:], in_=dnm_d.ap())
            d2nm = st.tile([P, NCHN], F32)
            nc.sync.dma_start(out=d2nm[:], in_=d2nm_d.ap())
            poolm = st.tile([P, NCHN * 3 * NUM_GRAPHS], BF16)
            nc.sync.dma_start(out=poolm[:], in_=pool_d.ap())
            wcat = st.tile([P, NW_MATS * CH], BF16)
            nc.sync.dma_start(out=wcat[:], in_=w_d.ap())
            bcat = st.tile([P, 1 + NUM_HIDDENS], F32)
            nc.sync.dma_start(out=bcat[:], in_=b_d.ap())
            bias_t = st.tile([P, NUM_GRAPHS], F32)
            nc.sync.dma_start(out=bias_t[:], in_=bias_d.ap())
            p1_t = st.tile([P, D_OUT_HID], F32)
            nc.sync.dma_start(out=p1_t[:], in_=p1_d.ap())
            p2_t = st.tile([D_OUT_HID, D_OUT], F32)
            nc.sync.dma_start(out=p2_t[:], in_=p2_d.ap())
            pb1_t = st.tile([D_OUT_HID, 1], F32)
            nc.sync.dma_start(out=pb1_t[:], in_=pb1_d.ap())
            pb2_t = st.tile([D_OUT, 1], F32)
            nc.sync.dma_start(out=pb2_t[:], in_=pb2_d.ap())

            H = feat.tile([P, NL], BF16, name="H")
            nc.sync.dma_start(out=H[:], in_=h0_d.ap())
            T1 = feat.tile([P, NL], BF16, name="T1")

            def spmv(table_ap, evict, chunk_writer=None, mid_ag=None):
                """One spmv pass over all windows; evict(w, psum_tile).

                chunk_writer(j0, j1) is invoked after every odd window to
                flush completed table chunks; mid_ag() is emitted before
                window MID_W's gathers so the piece-0 AllGather flies while
                the remaining windows' descriptors are generated.
                """
                soff = [0, 0]
                slot = 0
                for w in range(NW):
                    if mid_ag is not None and w == MID_W:
                        mid_ag()
                    vts = []
                    for h, (idx_t, v_pool) in enumerate(
                        ((idx_lo, vlo_p), (idx_hi, vhi_p))
                    ):
                        ck = int(K[w, :, h].sum())
                        if ck == 0:
                            vts.append(None)
                            continue
                        v = v_pool.tile([P, prep.Kmax * CH], BF16,
                                        tag=f"v{h}")
                        base = 0 if h == 0 else BASE_HI
                        nc.gpsimd.dma_gather(
                            out_ap=v[:, :ck * CH].rearrange(
                                "p (c e) -> p c e", e=CH),
                            in_ap=table_ap[base:base + 32768, :],
                            idxs_ap=idx_t[:, soff[h]:soff[h] + S[w, h]],
                            num_idxs=ck * 128,
                            num_idxs_reg=ck * 128,
                            elem_size=CH,
                            single_packet=SP,
                            queue_num=1 + ((2 * w + h) % 3),
                        )
                        vts.append(v)
                        soff[h] += int(S[w, h])
                    u = psu.tile([P, WW], F32, tag="u")
                    for s in range(SPW):
                        ntot_s = int(K[w, s, 0] + K[w, s, 1])
                        i = 0
                        for h in range(2):
                            coff = int(K[w, :s, h].sum())
                            for c in range(int(K[w, s, h])):
                                nc.tensor.matmul(
                                    out=u[:, s * SW:(s + 1) * SW],
                                    lhsT=vts[h][:, (coff + c) * CH:
                                                (coff + c + 1) * CH],
                                    rhs=m_all[:, slot * SW:(slot + 1) * SW],
                                    start=(i == 0),
                                    stop=(i == ntot_s - 1),
                                )
                                slot += 1
                                i += 1
                    evict(w, u)
                    if chunk_writer is not None and w % 2 == 1:
                        chunk_writer(7 * (w // 2), 7 * (w // 2) + 7)

            def write_chunks(src_cm, scale_nm, agi, j0, j1):
                """nm-transpose src_cm chunks [j0,j1) (scaled) -> agi.

                All chunks of the burst are scaled into one wide SBUF tile
                and written with a single DMA so the per-DMA fixed cost is
                amortized (one 32KB DMA per chunk serialized the pipeline
                at ~1.65us/chunk).
                """
                nb = j1 - j0
                ow = nm_p.tile([P, 7 * P], BF16, tag="ow")
                for c, j in enumerate(range(j0, j1)):
                    pt = pst.tile([P, P], BF16, tag="pt")
                    nc.tensor.transpose(pt[:], src_cm[:, j * P:(j + 1) * P],
                                        ident[:])
                    nc.vector.tensor_scalar(
                        out=ow[:, c * P:(c + 1) * P], in0=pt[:],
                        scalar1=scale_nm[:, j:j + 1],
                        scalar2=None, op0=ALU.mult)
                nc.sync.dma_start(
                    out=agi[j0 * P:j1 * P, :].rearrange(
                        "(c p) e -> p c e", p=P),
                    in_=ow[:, :nb * P].rearrange("p (c e) -> p c e", e=CH))

            def ag_full(agi, tbl):
                nc.gpsimd.collective_compute(
                    "AllGather", ALU.bypass, replica_groups=rg,
                    ins=[agi[:].opt()], outs=[tbl[:].opt()])

            n_layers = N_CONV
            tbl_cur = tbl0_d.ap()
            pg = psg.tile([P, 3 * NUM_GRAPHS], F32, name="pool")
            for l in range(n_layers):
                wofs = (0 if l == 0 else 3 + 3 * (l - 1)) * CH

                def ev1(w, u, l=l):
                    sl = slice(w * WW, (w + 1) * WW)
                    nc.vector.tensor_copy(out=T1[:, sl], in_=u[:, :WW])

                tblB = dram.tile([NTOT, CH], BF16, tag="tblB",
                                 name=f"tblB_{l}", addr_space="Shared")
                agiB = dram.tile([NL, CH], BF16, tag="agi", name=f"agiB_{l}")

                def cwB(j0, j1, agiB=agiB):
                    write_chunks(T1, d2nm, agiB, j0, j1)

                spmv(tbl_cur, ev1, chunk_writer=cwB)
                ag_full(agiB, tblB)

                def ev2(w, u, l=l, wofs=wofs):
                    sl = slice(w * WW, (w + 1) * WW)
                    t2 = nm_p.tile([P, WW], BF16, tag="t2")
                    nc.vector.tensor_tensor(out=t2[:], in0=u[:, :WW],
                                            in1=H[:, sl], op=ALU.subtract)
                    pw = psw.tile([P, WW], F32, tag="pw")
                    nc.tensor.matmul(out=pw[:], lhsT=wcat[:, wofs:wofs + CH],
                                     rhs=H[:, sl], start=True, stop=False)
                    nc.tensor.matmul(out=pw[:],
                                     lhsT=wcat[:, wofs + CH:wofs + 2 * CH],
                                     rhs=T1[:, sl], start=False, stop=False)
                    nc.tensor.matmul(out=pw[:],
                                     lhsT=wcat[:, wofs + 2 * CH:wofs + 3 * CH],
                                     rhs=t2[:], start=False, stop=True)
                    if l >= 1:
                        nc.scalar.activation(out=H[:, sl], in_=pw[:],
                                             func=AF.Silu,
                                             bias=bcat[:, l:l + 1], scale=1.0)
                    else:
                        nc.vector.tensor_scalar(
                            out=H[:, sl], in0=pw[:],
                            scalar1=bcat[:, l:l + 1], scalar2=None,
                            op0=ALU.add)

                if l < n_layers - 1:
                    tblA = dram.tile([NTOT, CH], BF16, tag="tblA",
                                     name=f"tblA_{l}", addr_space="Shared")
                    agiA = dram.tile([NL, CH], BF16, tag="agi2",
                                     name=f"agiA_{l}")

                    def cwA(j0, j1, agiA=agiA):
                        write_chunks(H, dnm, agiA, j0, j1)

                    spmv(tblB[:], ev2, chunk_writer=cwA)
                    ag_full(agiA, tblA)
                    tbl_cur = tblA[:]
                else:
                    # last conv pass: fold pooling in as the chunk writer
                    def pool_chunks(j0, j1):
                        for j in range(j0, j1):
                            pt = pst.tile([P, P], BF16, tag="pt")
                            nc.tensor.transpose(
                                pt[:], H[:, j * P:(j + 1) * P], ident[:])
                            hn = nm_p.tile([P, P], BF16, tag="o")
                            nc.vector.tensor_copy(out=hn[:], in_=pt[:])
                            nc.tensor.matmul(
                                out=pg[:], lhsT=hn[:],
                                rhs=poolm[:, j * 3 * NUM_GRAPHS:
                                          (j + 1) * 3 * NUM_GRAPHS],
                                start=(j == 0), stop=(j == NCHN - 1))

                    spmv(tblB[:], ev2, chunk_writer=pool_chunks)

            gsb = nm_p.tile([P, 3 * NUM_GRAPHS], BF16, tag="t2", name="gsb")
            nc.vector.tensor_copy(out=gsb[:], in_=pg[:])
            # combine: U = C0^T g0 + C1^T g1 + C2^T g2
            up = psw.tile([P, NUM_GRAPHS], F32, tag="pw", name="ucomb")
            for t in range(3):
                nc.tensor.matmul(
                    out=up[:],
                    lhsT=wcat[:, (12 + t) * CH:(13 + t) * CH],
                    rhs=gsb[:, t * NUM_GRAPHS:(t + 1) * NUM_GRAPHS],
                    start=(t == 0), stop=(t == 2))
            usb = nm_p.tile([P, NUM_GRAPHS], F32, tag="o", name="usb")
            nc.vector.tensor_copy(out=usb[:], in_=up[:])
            ar_in = dram.tile([P, NUM_GRAPHS], F32, tag="ar_in", name="ar_in")
            ar_out = dram.tile([P, NUM_GRAPHS], F32, tag="ar_out",
                               name="ar_out", addr_space="Shared")
            nc.sync.dma_start(out=ar_in[:], in_=usb[:])
            nc.gpsimd.collective_compute(
                "AllReduce", ALU.add, replica_groups=rg,
                ins=[ar_in[:].opt()], outs=[ar_out[:].opt()])
            gar = nm_p.tile([P, NUM_GRAPHS], F32, tag="t2", name="gar")
            nc.sync.dma_start(out=gar[:], in_=ar_out[:])
            gfull = nm_p.tile([P, NUM_GRAPHS], F32, tag="o", name="gfull")
            nc.vector.tensor_tensor(out=gfull[:], in0=gar[:], in1=bias_t[:],
                                    op=ALU.add)

            # ---- MLP (computed transposed)
            t1p = psg.tile([D_OUT_HID, NUM_GRAPHS], F32, tag="mp", name="m1")
            nc.tensor.matmul(out=t1p[:], lhsT=p1_t[:], rhs=gfull[:],
                             start=True, stop=True)
            s1 = nm_p.tile([D_OUT_HID, NUM_GRAPHS], F32, tag="t2", name="ms")
            nc.scalar.activation(out=s1[:], in_=t1p[:], func=AF.Relu,
                                 bias=pb1_t[:, 0:1], scale=1.0)
            t2p = psg.tile([D_OUT, NUM_GRAPHS], F32, tag="mp", name="m2")
            nc.tensor.matmul(out=t2p[:], lhsT=p2_t[:], rhs=s1[:],
                             start=True, stop=True)
            o2 = nm_p.tile([D_OUT, NUM_GRAPHS], F32, tag="o", name="mo")
            nc.vector.tensor_scalar(out=o2[:], in0=t2p[:],
                                    scalar1=pb2_t[:, 0:1], scalar2=None,
                                    op0=ALU.add)
            nc.sync.dma_start(out=out_d.ap(), in_=o2[:])

    nc.compile()
    return nc


def make_inputs(prep: Prep, x, W1, b1, Wh, bh, W2, b2, P1, pb1, P2, pb2,
                batch):
    import scipy.sparse as sp

    N = prep.N
    dinv = prep.dinv
    np_bf = mybir.dt.np(BF16)

    x = np.asarray(x, np.float32)
    batch = np.asarray(batch, np.int64)
    W2 = np.asarray(W2, np.float32)
    b2 = np.asarray(b2, np.float32)
    # permuted full arrays (padded)
    xp = np.zeros((NTOT, CH), np.float32)
    xp[prep.pos[:N]] = x
    dinv_p = np.zeros(NTOT, np.float32)
    dinv_p[prep.pos[:N]] = dinv
    batch_p = np.full(NTOT, -1, np.int64)
    batch_p[prep.pos[:N]] = batch

    # tbl0 is laid out in table-row order (trow relabeling)
    tbl0 = np.zeros((NTOT, CH), np.float32)
    tbl0[prep.trow] = xp * dinv_p[:, None]
    tbl0 = tbl0.astype(np_bf)

    # pooled rows: B (one-hot), BS = B@S, BS2 = BS@S  (original node ids)
    w_e = (-dinv[prep.src] * dinv[prep.dst]).astype(np.float64)
    Smat = sp.coo_matrix((w_e, (prep.dst, prep.src)), shape=(N, N)).tocsr()
    Bmat = sp.coo_matrix((np.ones(N), (batch, np.arange(N))),
                         shape=(NUM_GRAPHS, N)).tocsr()
    BS = np.asarray((Bmat @ Smat).todense())
    BS2 = BS @ Smat  # dense @ sparse -> dense [64, N]
    # permute+pad to [64, NTOT]
    Bp = np.zeros((NUM_GRAPHS, NTOT), np.float32)
    BSp = np.zeros((NUM_GRAPHS, NTOT), np.float32)
    BS2p = np.zeros((NUM_GRAPHS, NTOT), np.float32)
    Bp[batch_p[prep.pos[:N]], prep.pos[:N]] = 1.0
    BSp[:, prep.pos[:N]] = BS
    BS2p[:, prep.pos[:N]] = BS2

    cnt = np.bincount(batch, minlength=NUM_GRAPHS).astype(np.float32)
    bias_mat = np.outer(b2, cnt).astype(np.float32)  # [128, 64]

    # conv weights (12) + pool-combo (3): C0=W0-W2, C1=W1, C2=2*W2
    wcat = np.concatenate(
        [W1[k] for k in range(3)]
        + [Wh[i][k] for i in range(NUM_HIDDENS) for k in range(3)]
        + [W2[0] - W2[2], W2[1], 2.0 * W2[2]], axis=1).astype(np_bf)
    bcat = np.stack([b1] + [bh[i] for i in range(NUM_HIDDENS)],
                    axis=1).astype(np.float32)

    in_maps = []
    for k in range(NCORES):
        blk = slice(k * NL, (k + 1) * NL)
        d_blk = dinv_p[blk]
        h0 = np.ascontiguousarray(xp[blk].T).astype(np_bf)  # [128, NL]
        d_nm = d_blk.reshape(NCHN, P).T.astype(np.float32)
        d2_nm = (2.0 * d_blk).reshape(NCHN, P).T.astype(np.float32)
        # poolmat node-major: [128, NCHN*192]; chunk j cols: [B | BS | BS2]
        pm = np.zeros((P, NCHN * 3 * NUM_GRAPHS), np.float32)
        for j in range(NCHN):
            nsl = slice(k * NL + j * P, k * NL + (j + 1) * P)
            o = j * 3 * NUM_GRAPHS
            pm[:, o:o + NUM_GRAPHS] = Bp[:, nsl].T
            pm[:, o + NUM_GRAPHS:o + 2 * NUM_GRAPHS] = BSp[:, nsl].T
            pm[:, o + 2 * NUM_GRAPHS:o + 3 * NUM_GRAPHS] = BS2p[:, nsl].T
        in_maps.append(dict(
            tbl0=tbl0,
            h0=h0,
            idx_lo=prep.idx_in[k][0],
            idx_hi=prep.idx_in[k][1],
            m_all=prep.m_in[k].astype(np_bf),
            dinv_nm=np.ascontiguousarray(d_nm),
            dinv2_nm=np.ascontiguousarray(d2_nm),
            poolmat=pm.astype(np_bf),
            Wcat=np.ascontiguousarray(wcat),
            bcat=np.ascontiguousarray(bcat),
            bias_mat=bias_mat,
            P1=np.asarray(P1, np.float32),
            P2=np.asarray(P2, np.float32),
            pb1=np.asarray(pb1, np.float32)[:, None],
            pb2=np.asarray(pb2, np.float32)[:, None],
        ))
    return in_maps


_CACHE = {}


def kernel(x, edge_index, batch, W1, b1, Wh, bh, W2, b2, P1, pb1, P2, pb2):
    import os as _os
    from concourse import bass_utils as _bu

    x = np.asarray(x, dtype=np.float32)
    edge_index = np.asarray(edge_index)
    batch = np.asarray(batch)
    key = (x.shape, edge_index.shape)
    if _CACHE.get("key") != key:
        prep = Prep(x.shape[0], edge_index)
        nc = build_kernel(prep)
        _CACHE.update(key=key, prep=prep, nc=nc)
    prep, nc = _CACHE["prep"], _CACHE["nc"]
    in_maps = make_inputs(prep, x, W1, b1, Wh, bh, W2, b2, P1, pb1, P2, pb2,
                          batch)
    trace = bool(int(_os.environ.get("CHEB_TRACE", "0")))
    res = _bu.run_bass_kernel_spmd(nc, in_maps, core_ids=list(range(NCORES)),
                                   trace=trace)
    _CACHE["res"] = res
    if res.exec_time_ns is not None:
        print(f"HW exec time: {res.exec_time_ns} ns")
    out = res.results[0]["out"]  # [16, 64]
    return np.ascontiguousarray(out.T).astype(np.float32)


# revision 3
# speedup vs baseline: 1.0082x; 1.0082x over previous
"""ChebConvNet (K=3, 5 conv layers + pool + MLP) on 8 TRN2 NeuronCores — v4.

v2 base (bf16 datapath; host scatter one-hots; serpentine balance; lo/hi
overlap split; pipelined dma_gather + PE scatter matmuls) plus:
 - last conv layer + pool folded into precomputed pooled rows B/BS/BS2
   (g = (B h)(W0-W2) + (BS h)W1 + (BS2 h)(2W2) + cnt*b2): removes 2 spmv
   passes and 2 AllGathers.
 - split AllGather: table rows are relabeled so each rank's local rows
   [0,NPIECE0) form a contiguous leading block of the table; the AG for
   that block fires mid-pass (hidden under remaining gather descriptor
   generation) and only the small tail AG is exposed.
 - incremental table writes: transpose/scale/DMA of table chunks runs
   every 2 windows inside the pass instead of as a post-pass phase.
 - pool phase folded into the last pass the same way (per-chunk
   transpose+pool matmul as H windows complete).
"""
import numpy as np
import concourse.bacc as bacc
import concourse.bass as bass
import concourse.mybir as mybir
import concourse.tile as tile
from concourse.masks import make_identity

F32 = mybir.dt.float32
BF16 = mybir.dt.bfloat16
I16 = mybir.dt.int16
AF = mybir.ActivationFunctionType
ALU = mybir.AluOpType

P = 128
CH = 128
NUM_HIDDENS = 3
NUM_GRAPHS = 64
D_OUT_HID = 32
D_OUT = 16
NCORES = 8

NL = 6272            # padded nodes per core (98 subwindows of 64)
NTOT = NL * NCORES   # 50176 padded total
SW = 64              # subwindow width (dst cols per scatter matmul)
SPW = 7              # subwindows per PSUM window
WW = SW * SPW        # 448 dst per window
NSUB = NL // SW      # 98
NW = NSUB // SPW     # 14 windows
NCHN = NL // P       # 49 node-major chunks per core
BASE_HI = NTOT - 32768  # 17408; lo rows [0,32768), hi rows [17408, NTOT)

N_CONV = 1 + NUM_HIDDENS      # device conv layers (last conv folded into pool)
# split-AG: table rows are rank-major within two pieces; piece 0 holds each
# rank's local rows [0, NPIECE0) (= windows 0..9), piece 1 the rest.
NPIECE0 = NL                  # single-piece AG (split-AG regressed: Local
NPIECE1 = 0                   # collective output runs at ~half Shared BW)
T0TOT = NPIECE0 * NCORES
MID_W = None                  # no mid-pass AG

import os as _os_mod
SP = bool(int(_os_mod.environ.get("CHEB_SP", "0")))  # dma_gather single_packet


def cdiv(a, b):
    return (a + b - 1) // b


class Prep:
    """Host-side: permutation, edge bucketing, idx streams, m matrices."""

    def __init__(self, n_nodes, edge_index):
        N = self.N = n_nodes
        src = np.asarray(edge_index[0], dtype=np.int64)
        dst = np.asarray(edge_index[1], dtype=np.int64)
        keep = src != dst
        src, dst = src[keep], dst[keep]
        self.src, self.dst = src, dst
        deg = np.bincount(src, minlength=N).astype(np.float64)
        self.dinv = np.where(deg > 0, 1.0 / np.sqrt(np.maximum(deg, 1.0)),
                             0.0).astype(np.float32)

        # ---- serpentine in-degree balancing over 784 buckets of 64 slots
        in_deg = np.zeros(NTOT, np.int64)
        in_deg[:N] = np.bincount(dst, minlength=N)
        order = np.argsort(-in_deg, kind="stable")  # pads (deg 0) at end
        NB = NCORES * NSUB  # 784 buckets
        pos = np.empty(NTOT, np.int64)
        for r in range(SW):
            sl = order[r * NB:(r + 1) * NB]
            b = np.arange(NB) if r % 2 == 0 else np.arange(NB)[::-1]
            pos[sl] = (b % NCORES) * NL + (b // NCORES) * SW + r
        self.pos = pos  # old id -> new id

        # table-row relabeling: position (core k, local row r) -> table row
        # k*NPIECE0 + r for r < NPIECE0, else T0TOT + k*NPIECE1 + (r-NPIECE0).
        # Makes each AG piece a contiguous table block while keeping the
        # dst-side (window/subwindow) layout untouched.
        pidx = np.arange(NTOT)
        rr, kk = pidx % NL, pidx // NL
        self.trow = np.where(rr < NPIECE0, kk * NPIECE0 + rr,
                             T0TOT + kk * NPIECE1 + (rr - NPIECE0))

        ps = self.trow[pos[src]]
        pd = pos[dst]
        core = pd // NL
        rem = pd % NL
        sub = rem // SW
        dcol = rem % SW
        wdst = -self.dinv[dst]  # m value

        # ---- half split: fixed lo (<BASE_HI), fixed hi (>=32768),
        # flexible in [BASE_HI, 32768) balanced per (core, sub) bucket
        half = np.where(ps < BASE_HI, 0, np.where(ps >= 32768, 1, -1))
        bucket_id = core * NSUB + sub
        bord = np.argsort(bucket_id, kind="stable")
        bounds = np.searchsorted(bucket_id[bord], np.arange(NB + 1))
        for b in range(NB):
            seg = bord[bounds[b]:bounds[b + 1]]
            if len(seg) == 0:
                continue
            hm = half[seg]
            nlo = int((hm == 0).sum())
            flex = seg[hm == -1]
            tot = len(seg)
            want_lo = max(0, min(len(flex), (tot + 1) // 2 - nlo))
            half[flex[:want_lo]] = 0
            half[flex[want_lo:]] = 1

        w = sub // SPW
        s_in_w = sub % SPW
        cnt = np.zeros((NCORES, NW, SPW, 2), np.int64)
        np.add.at(cnt, (core, w, s_in_w, half), 1)
        K = cdiv(cnt.max(axis=0), 128)  # [NW, SPW, 2]
        K[:, :, 0] = np.maximum(K[:, :, 0], 1)
        self.K = K
        self.S = K.sum(axis=1) * 128 // 16  # [NW, 2] idx cols per call
        self.Stot = self.S.sum(axis=0)      # [2]
        self.nslot = int(K.sum())
        self.Kmax = int(K.sum(axis=1).max())

        # slot order: w asc, s asc, h asc, c asc (must match device loops)
        slot_of = np.zeros((NW, SPW, 2), np.int64)
        t = 0
        for wi in range(NW):
            for si in range(SPW):
                for hi in range(2):
                    slot_of[wi, si, hi] = t
                    t += int(K[wi, si, hi])

        self.idx_in = []
        self.m_in = []
        for k in range(NCORES):
            msk = core == k
            kh, kw, kss, kd, kps, kwd = (half[msk], w[msk], s_in_w[msk],
                                         dcol[msk], ps[msk], wdst[msk])
            idx_h = [np.zeros((P, max(int(self.Stot[h]), 1)), np.int16)
                     for h in range(2)]
            m_all = np.zeros((P, self.nslot * SW), np.float32)
            soff = [0, 0]
            for wi in range(NW):
                for h in range(2):
                    stream = []
                    for si in range(SPW):
                        emsk = (kw == wi) & (kss == si) & (kh == h)
                        e_d = kd[emsk]
                        e_ps = kps[emsk]
                        e_wd = kwd[emsk]
                        kk = int(K[wi, si, h])
                        npad = kk * 128 - len(e_ps)
                        assert npad >= 0, (wi, si, h, len(e_ps))
                        base = 0 if h == 0 else BASE_HI
                        stream.append(np.concatenate(
                            [e_ps - base, np.zeros(npad, np.int64)]))
                        sl0 = slot_of[wi, si, h]
                        i = np.arange(len(e_d))
                        np.add.at(m_all, (i % 128,
                                          (sl0 + i // 128) * SW + e_d), e_wd)
                    st = np.concatenate(stream)
                    S_w = int(self.S[wi, h])
                    if S_w:
                        wrapped = st.reshape(S_w, 16).T.astype(np.int16)
                        idx_h[h][:, soff[h]:soff[h] + S_w] = np.tile(
                            wrapped, (8, 1))
                    soff[h] += S_w
            self.idx_in.append(idx_h)
            self.m_in.append(m_all)


def build_kernel(prep: Prep):
    NW_MATS = 3 + NUM_HIDDENS * 3 + 3  # 12 conv + 3 pool-combo
    K, S = prep.K, prep.S
    nc = bacc.Bacc("TRN2", target_bir_lowering=False, debug=False,
                   num_devices=NCORES, num_swdge_queues=4)
    rg = [list(range(NCORES))]

    tbl0_d = nc.dram_tensor("tbl0", [NTOT, CH], BF16, kind="ExternalInput")
    h0_d = nc.dram_tensor("h0", [P, NL], BF16, kind="ExternalInput")
    idx_lo_d = nc.dram_tensor("idx_lo", [P, max(prep.Stot[0], 1)], I16,
                              kind="ExternalInput")
    idx_hi_d = nc.dram_tensor("idx_hi", [P, max(prep.Stot[1], 1)], I16,
                              kind="ExternalInput")
    m_d = nc.dram_tensor("m_all", [P, prep.nslot * SW], BF16,
                         kind="ExternalInput")
    dnm_d = nc.dram_tensor("dinv_nm", [P, NCHN], F32, kind="ExternalInput")
    d2nm_d = nc.dram_tensor("dinv2_nm", [P, NCHN], F32, kind="ExternalInput")
    pool_d = nc.dram_tensor("poolmat", [P, NCHN * 3 * NUM_GRAPHS], BF16,
                            kind="ExternalInput")
    w_d = nc.dram_tensor("Wcat", [P, NW_MATS * CH], BF16,
                         kind="ExternalInput")
    b_d = nc.dram_tensor("bcat", [P, 1 + NUM_HIDDENS], F32,
                         kind="ExternalInput")
    bias_d = nc.dram_tensor("bias_mat", [P, NUM_GRAPHS], F32,
                            kind="ExternalInput")
    p1_d = nc.dram_tensor("P1", [P, D_OUT_HID], F32, kind="ExternalInput")
    p2_d = nc.dram_tensor("P2", [D_OUT_HID, D_OUT], F32, kind="ExternalInput")
    pb1_d = nc.dram_tensor("pb1", [D_OUT_HID, 1], F32, kind="ExternalInput")
    pb2_d = nc.dram_tensor("pb2", [D_OUT, 1], F32, kind="ExternalInput")
    out_d = nc.dram_tensor("out", [D_OUT, NUM_GRAPHS], F32,
                           kind="ExternalOutput")

    with tile.TileContext(nc) as tc:
        with (
            tc.tile_pool(name="static", bufs=1) as st,
            tc.tile_pool(name="feat", bufs=1) as feat,
            tc.tile_pool(name="vlo", bufs=4) as vlo_p,
            tc.tile_pool(name="vhi", bufs=4) as vhi_p,
            tc.tile_pool(name="nm", bufs=2) as nm_p,
            tc.tile_pool(name="psu", bufs=2, space="PSUM") as psu,
            tc.tile_pool(name="psw", bufs=2, space="PSUM") as psw,
            tc.tile_pool(name="pst", bufs=2, space="PSUM") as pst,
            tc.tile_pool(name="psg", bufs=1, space="PSUM") as psg,
            tc.tile_pool(name="dram", bufs=1, space="DRAM") as dram,
        ):
            idx_lo = st.tile([P, max(prep.Stot[0], 1)], I16)
            nc.sync.dma_start(out=idx_lo[:], in_=idx_lo_d.ap())
            idx_hi = st.tile([P, max(prep.Stot[1], 1)], I16)
            nc.sync.dma_start(out=idx_hi[:], in_=idx_hi_d.ap())
            ident = st.tile([P, P], BF16)
            make_identity(nc, ident[:])
            m_all = st.tile([P, prep.nslot * SW], BF16)
            nc.sync.dma_start(out=m_all[:], in_=m_d.ap())
            dnm = st.tile([P, NCHN], F32)
            nc.sync.dma_start(out=dnm[:], in_=dnm_d.ap())
            d2nm = st.tile([P, NCHN], F32)
            nc.sync.dma_start(out=d2nm[:], in_=d2nm_d.ap())
            poolm = st.tile([P, NCHN * 3 * NUM_GRAPHS], BF16)
            nc.sync.dma_start(out=poolm[:], in_=pool_d.ap())
            wcat = st.tile([P, NW_MATS * CH], BF16)
            nc.sync.dma_start(out=wcat[:], in_=w_d.ap())
            bcat = st.tile([P, 1 + NUM_HIDDENS], F32)
            nc.sync.dma_start(out=bcat[:], in_=b_d.ap())
            bias_t = st.tile([P, NUM_GRAPHS], F32)
            nc.sync.dma_start(out=bias_t[:], in_=bias_d.ap())
            p1_t = st.tile([P, D_OUT_HID], F32)
            nc.sync.dma_start(out=p1_t[:], in_=p1_d.ap())
            p2_t = st.tile([D_OUT_HID, D_OUT], F32)
            nc.sync.dma_start(out=p2_t[:], in_=p2_d.ap())
            pb1_t = st.tile([D_OUT_HID, 1], F32)
            nc.sync.dma_start(out=pb1_t[:], in_=pb1_d.ap())
            pb2_t = st.tile([D_OUT, 1], F32)
            nc.sync.dma_start(out=pb2_t[:], in_=pb2_d.ap())

            H = feat.tile([P, NL], BF16, name="H")
            nc.sync.dma_start(out=H[:], in_=h0_d.ap())
            T1 = feat.tile([P, NL], BF16, name="T1")

            # warmup collectives: the first AllGather/AllReduce pay ~15-20us
            # of ring/ENCD cold-start; fire tiny dummies here so that cost
            # overlaps pass-1's gather generation instead of boundary 1.
            wu_in = dram.tile([NCORES, CH], BF16, name="wu_in")
            wu_out = dram.tile([NCORES * NCORES, CH], BF16, name="wu_out",
                               addr_space="Shared")
            nc.gpsimd.collective_compute(
                "AllGather", ALU.bypass, replica_groups=rg,
                ins=[wu_in[:].opt()], outs=[wu_out[:].opt()])
            wr_in = dram.tile([P, 1], F32, name="wr_in")
            wr_out = dram.tile([P, 1], F32, name="wr_out",
                               addr_space="Shared")
            nc.gpsimd.collective_compute(
                "AllReduce", ALU.add, replica_groups=rg,
                ins=[wr_in[:].opt()], outs=[wr_out[:].opt()])

            def spmv(table_ap, evict, chunk_writer=None, mid_ag=None):
                """One spmv pass over all windows; evict(w, psum_tile).

                chunk_writer(j0, j1) is invoked after every odd window to
                flush completed table chunks; mid_ag() is emitted before
                window MID_W's gathers so the piece-0 AllGather flies while
                the remaining windows' descriptors are generated.
                """
                soff = [0, 0]
                slot = 0
                for w in range(NW):
                    if mid_ag is not None and w == MID_W:
                        mid_ag()
                    vts = []
                    for h, (idx_t, v_pool) in enumerate(
                        ((idx_lo, vlo_p), (idx_hi, vhi_p))
                    ):
                        ck = int(K[w, :, h].sum())
                        if ck == 0:
                            vts.append(None)
                            continue
                        v = v_pool.tile([P, prep.Kmax * CH], BF16,
                                        tag=f"v{h}")
                        base = 0 if h == 0 else BASE_HI
                        nc.gpsimd.dma_gather(
                            out_ap=v[:, :ck * CH].rearrange(
                                "p (c e) -> p c e", e=CH),
                            in_ap=table_ap[base:base + 32768, :],
                            idxs_ap=idx_t[:, soff[h]:soff[h] + S[w, h]],
                            num_idxs=ck * 128,
                            num_idxs_reg=ck * 128,
                            elem_size=CH,
                            single_packet=SP,
                            queue_num=1 + ((2 * w + h) % 3),
                        )
                        vts.append(v)
                        soff[h] += int(S[w, h])
                    u = psu.tile([P, WW], F32, tag="u")
                    for s in range(SPW):
                        ntot_s = int(K[w, s, 0] + K[w, s, 1])
                        i = 0
                        for h in range(2):
                            coff = int(K[w, :s, h].sum())
                            for c in range(int(K[w, s, h])):
                                nc.tensor.matmul(
                                    out=u[:, s * SW:(s + 1) * SW],
                                    lhsT=vts[h][:, (coff + c) * CH:
                                                (coff + c + 1) * CH],
                                    rhs=m_all[:, slot * SW:(slot + 1) * SW],
                                    start=(i == 0),
                                    stop=(i == ntot_s - 1),
                                )
                                slot += 1
                                i += 1
                    evict(w, u)
                    if chunk_writer is not None and w % 2 == 1:
                        chunk_writer(7 * (w // 2), 7 * (w // 2) + 7)

            def write_chunks(src_cm, scale_nm, agi, j0, j1):
                """nm-transpose src_cm chunks [j0,j1) (scaled) -> agi.

                All chunks of the burst are scaled into one wide SBUF tile
                and written with a single DMA so the per-DMA fixed cost is
                amortized (one 32KB DMA per chunk serialized the pipeline
                at ~1.65us/chunk).
                """
                nb = j1 - j0
                ow = nm_p.tile([P, 7 * P], BF16, tag="ow")
                for c, j in enumerate(range(j0, j1)):
                    pt = pst.tile([P, P], BF16, tag="pt")
                    nc.tensor.transpose(pt[:], src_cm[:, j * P:(j + 1) * P],
                                        ident[:])
                    nc.vector.tensor_scalar(
                        out=ow[:, c * P:(c + 1) * P], in0=pt[:],
                        scalar1=scale_nm[:, j:j + 1],
                        scalar2=None, op0=ALU.mult)
                nc.sync.dma_start(
                    out=agi[j0 * P:j1 * P, :].rearrange(
                        "(c p) e -> p c e", p=P),
                    in_=ow[:, :nb * P].rearrange("p (c e) -> p c e", e=CH))

            def ag_full(agi, tbl):
                nc.gpsimd.collective_compute(
                    "AllGather", ALU.bypass, replica_groups=rg,
                    ins=[agi[:].opt()], outs=[tbl[:].opt()])

            n_layers = N_CONV
            tbl_cur = tbl0_d.ap()
            pg = psg.tile([P, 3 * NUM_GRAPHS], F32, name="pool")
            for l in range(n_layers):
                wofs = (0 if l == 0 else 3 + 3 * (l - 1)) * CH

                def ev1(w, u, l=l):
                    sl = slice(w * WW, (w + 1) * WW)
                    nc.vector.tensor_copy(out=T1[:, sl], in_=u[:, :WW])

                tblB = dram.tile([NTOT, CH], BF16, tag="tblB",
                                 name=f"tblB_{l}", addr_space="Shared")
                agiB = dram.tile([NL, CH], BF16, tag="agi", name=f"agiB_{l}")

                def cwB(j0, j1, agiB=agiB):
                    write_chunks(T1, d2nm, agiB, j0, j1)

                spmv(tbl_cur, ev1, chunk_writer=cwB)
                ag_full(agiB, tblB)

                def ev2(w, u, l=l, wofs=wofs):
                    sl = slice(w * WW, (w + 1) * WW)
                    t2 = nm_p.tile([P, WW], BF16, tag="t2")
                    nc.vector.tensor_tensor(out=t2[:], in0=u[:, :WW],
                                            in1=H[:, sl], op=ALU.subtract)
                    pw = psw.tile([P, WW], F32, tag="pw")
                    nc.tensor.matmul(out=pw[:], lhsT=wcat[:, wofs:wofs + CH],
                                     rhs=H[:, sl], start=True, stop=False)
                    nc.tensor.matmul(out=pw[:],
                                     lhsT=wcat[:, wofs + CH:wofs + 2 * CH],
                                     rhs=T1[:, sl], start=False, stop=False)
                    nc.tensor.matmul(out=pw[:],
                                     lhsT=wcat[:, wofs + 2 * CH:wofs + 3 * CH],
                                     rhs=t2[:], start=False, stop=True)
                    if l >= 1:
                        nc.scalar.activation(out=H[:, sl], in_=pw[:],
                                             func=AF.Silu,
                                             bias=bcat[:, l:l + 1], scale=1.0)
                    else:
                        nc.vector.tensor_scalar(
                            out=H[:, sl], in0=pw[:],
                            scalar1=bcat[:, l:l + 1], scalar2=None,
                            op0=ALU.add)

                if l < n_layers - 1:
                    tblA = dram.tile([NTOT, CH], BF16, tag="tblA",
                                     name=f"tblA_{l}", addr_space="Shared")
                    agiA = dram.tile([NL, CH], BF16, tag="agi2",
                                     name=f"agiA_{l}")

                    def cwA(j0, j1, agiA=agiA):
                        write_chunks(H, dnm, agiA, j0, j1)

                    spmv(tblB[:], ev2, chunk_writer=cwA)
                    ag_full(agiA, tblA)
                    tbl_cur = tblA[:]
                else:
                    # last conv pass: fold pooling in as the chunk writer
                    def pool_chunks(j0, j1):
                        for j in range(j0, j1):
                            pt = pst.tile([P, P], BF16, tag="pt")
                            nc.tensor.transpose(
                                pt[:], H[:, j * P:(j + 1) * P], ident[:])
                            hn = nm_p.tile([P, P], BF16, tag="o")
                            nc.vector.tensor_copy(out=hn[:], in_=pt[:])
                            nc.tensor.matmul(
                                out=pg[:], lhsT=hn[:],
                                rhs=poolm[:, j * 3 * NUM_GRAPHS:
                                          (j + 1) * 3 * NUM_GRAPHS],
                                start=(j == 0), stop=(j == NCHN - 1))

                    spmv(tblB[:], ev2, chunk_writer=pool_chunks)

            gsb = nm_p.tile([P, 3 * NUM_GRAPHS], BF16, tag="t2", name="gsb")
            nc.vector.tensor_copy(out=gsb[:], in_=pg[:])
            # combine: U = C0^T g0 + C1^T g1 + C2^T g2
            up = psw.tile([P, NUM_GRAPHS], F32, tag="pw", name="ucomb")
            for t in range(3):
                nc.tensor.matmul(
                    out=up[:],
                    lhsT=wcat[:, (12 + t) * CH:(13 + t) * CH],
                    rhs=gsb[:, t * NUM_GRAPHS:(t + 1) * NUM_GRAPHS],
                    start=(t == 0), stop=(t == 2))
            usb = nm_p.tile([P, NUM_GRAPHS], F32, tag="o", name="usb")
            nc.vector.tensor_copy(out=usb[:], in_=up[:])
            ar_in = dram.tile([P, NUM_GRAPHS], F32, tag="ar_in", name="ar_in")
            ar_out = dram.tile([P, NUM_GRAPHS], F32, tag="ar_out",
                               name="ar_out", addr_space="Shared")
            nc.sync.dma_start(out=ar_in[:], in_=usb[:])
            nc.gpsimd.collective_compute(
                "AllReduce", ALU.add, replica_groups=rg,
                ins=[ar_in[:].opt()], outs=[ar_out[:].opt()])
            gar = nm_p.tile([P, NUM_GRAPHS], F32, tag="t2", name="gar")
            nc.sync.dma_start(out=gar[:], in_=ar_out[:])
            gfull = nm_p.tile([P, NUM_GRAPHS], F32, tag="o", name="gfull")
            nc.vector.tensor_tensor(out=gfull[:], in0=gar[:], in1=bias_t[:],
                                    op=ALU.add)

            # ---- MLP (computed transposed)
            t1p = psg.tile([D_OUT_HID, NUM_GRAPHS], F32, tag="mp", name="m1")
            nc.tensor.matmul(out=t1p[:], lhsT=p1_t[:], rhs=gfull[:],
                             start=True, stop=True)
            s1 = nm_p.tile([D_OUT_HID, NUM_GRAPHS], F32, tag="t2", name="ms")
            nc.scalar.activation(out=s1[:], in_=t1p[:], func=AF.Relu,
                                 bias=pb1_t[:, 0:1], scale=1.0)
            t2p = psg.tile([D_OUT, NUM_GRAPHS], F32, tag="mp", name="m2")
            nc.tensor.matmul(out=t2p[:], lhsT=p2_t[:], rhs=s1[:],
                             start=True, stop=True)
            o2 = nm_p.tile([D_OUT, NUM_GRAPHS], F32, tag="o", name="mo")
            nc.vector.tensor_scalar(out=o2[:], in0=t2p[:],
                                    scalar1=pb2_t[:, 0:1], scalar2=None,
                                    op0=ALU.add)
            nc.sync.dma_start(out=out_d.ap(), in_=o2[:])

    nc.compile()
    return nc


def make_inputs(prep: Prep, x, W1, b1, Wh, bh, W2, b2, P1, pb1, P2, pb2,
                batch):
    import scipy.sparse as sp

    N = prep.N
    dinv = prep.dinv
    np_bf = mybir.dt.np(BF16)

    x = np.asarray(x, np.float32)
    batch = np.asarray(batch, np.int64)
    W2 = np.asarray(W2, np.float32)
    b2 = np.asarray(b2, np.float32)
    # permuted full arrays (padded)
    xp = np.zeros((NTOT, CH), np.float32)
    xp[prep.pos[:N]] = x
    dinv_p = np.zeros(NTOT, np.float32)
    dinv_p[prep.pos[:N]] = dinv
    batch_p = np.full(NTOT, -1, np.int64)
    batch_p[prep.pos[:N]] = batch

    # tbl0 is laid out in table-row order (trow relabeling)
    tbl0 = np.zeros((NTOT, CH), np.float32)
    tbl0[prep.trow] = xp * dinv_p[:, None]
    tbl0 = tbl0.astype(np_bf)

    # pooled rows: B (one-hot), BS = B@S, BS2 = BS@S  (original node ids)
    w_e = (-dinv[prep.src] * dinv[prep.dst]).astype(np.float64)
    Smat = sp.coo_matrix((w_e, (prep.dst, prep.src)), shape=(N, N)).tocsr()
    Bmat = sp.coo_matrix((np.ones(N), (batch, np.arange(N))),
                         shape=(NUM_GRAPHS, N)).tocsr()
    BS = np.asarray((Bmat @ Smat).todense())
    BS2 = BS @ Smat  # dense @ sparse -> dense [64, N]
    # permute+pad to [64, NTOT]
    Bp = np.zeros((NUM_GRAPHS, NTOT), np.float32)
    BSp = np.zeros((NUM_GRAPHS, NTOT), np.float32)
    BS2p = np.zeros((NUM_GRAPHS, NTOT), np.float32)
    Bp[batch_p[prep.pos[:N]], prep.pos[:N]] = 1.0
    BSp[:, prep.pos[:N]] = BS
    BS2p[:, prep.pos[:N]] = BS2

    cnt = np.bincount(batch, minlength=NUM_GRAPHS).astype(np.float32)
    bias_mat = np.outer(b2, cnt).astype(np.float32)  # [128, 64]

    # conv weights (12) + pool-combo (3): C0=W0-W2, C1=W1, C2=2*W2
    wcat = np.concatenate(
        [W1[k] for k in range(3)]
        + [Wh[i][k] for i in range(NUM_HIDDENS) for k in range(3)]
        + [W2[0] - W2[2], W2[1], 2.0 * W2[2]], axis=1).astype(np_bf)
    bcat = np.stack([b1] + [bh[i] for i in range(NUM_HIDDENS)],
                    axis=1).astype(np.float32)

    in_maps = []
    for k in range(NCORES):
        blk = slice(k * NL, (k + 1) * NL)
        d_blk = dinv_p[blk]
        h0 = np.ascontiguousarray(xp[blk].T).astype(np_bf)  # [128, NL]
        d_nm = d_blk.reshape(NCHN, P).T.astype(np.float32)
        d2_nm = (2.0 * d_blk).reshape(NCHN, P).T.astype(np.float32)
        # poolmat node-major: [128, NCHN*192]; chunk j cols: [B | BS | BS2]
        pm = np.zeros((P, NCHN * 3 * NUM_GRAPHS), np.float32)
        for j in range(NCHN):
            nsl = slice(k * NL + j * P, k * NL + (j + 1) * P)
            o = j * 3 * NUM_GRAPHS
            pm[:, o:o + NUM_GRAPHS] = Bp[:, nsl].T
            pm[:, o + NUM_GRAPHS:o + 2 * NUM_GRAPHS] = BSp[:, nsl].T
            pm[:, o + 2 * NUM_GRAPHS:o + 3 * NUM_GRAPHS] = BS2p[:, nsl].T
        in_maps.append(dict(
            tbl0=tbl0,
            h0=h0,
            idx_lo=prep.idx_in[k][0],
            idx_hi=prep.idx_in[k][1],
            m_all=prep.m_in[k].astype(np_bf),
            dinv_nm=np.ascontiguousarray(d_nm),
            dinv2_nm=np.ascontiguousarray(d2_nm),
            poolmat=pm.astype(np_bf),
            Wcat=np.ascontiguousarray(wcat),
            bcat=np.ascontiguousarray(bcat),
            bias_mat=bias_mat,
            P1=np.asarray(P1, np.float32),
            P2=np.asarray(P2, np.float32),
            pb1=np.asarray(pb1, np.float32)[:, None],
            pb2=np.asarray(pb2, np.float32)[:, None],
        ))
    return in_maps


_CACHE = {}


def kernel(x, edge_index, batch, W1, b1, Wh, bh, W2, b2, P1, pb1, P2, pb2):
    import os as _os
    from concourse import bass_utils as _bu

    x = np.asarray(x, dtype=np.float32)
    edge_index = np.asarray(edge_index)
    batch = np.asarray(batch)
    key = (x.shape, edge_index.shape)
    if _CACHE.get("key") != key:
        prep = Prep(x.shape[0], edge_index)
        nc = build_kernel(prep)
        _CACHE.update(key=key, prep=prep, nc=nc)
    prep, nc = _CACHE["prep"], _CACHE["nc"]
    in_maps = make_inputs(prep, x, W1, b1, Wh, bh, W2, b2, P1, pb1, P2, pb2,
                          batch)
    trace = bool(int(_os.environ.get("CHEB_TRACE", "0")))
    res = _bu.run_bass_kernel_spmd(nc, in_maps, core_ids=list(range(NCORES)),
                                   trace=trace)
    _CACHE["res"] = res
    if res.exec_time_ns is not None:
        print(f"HW exec time: {res.exec_time_ns} ns")
    out = res.results[0]["out"]  # [16, 64]
    return np.ascontiguousarray(out.T).astype(np.float32)


# revision 4
# speedup vs baseline: 1.1787x; 1.1691x over previous
"""ChebConvNet (K=3, 5 conv layers + pool + MLP) on 8 TRN2 NeuronCores — v4.

v2 base (bf16 datapath; host scatter one-hots; serpentine balance; lo/hi
overlap split; pipelined dma_gather + PE scatter matmuls) plus:
 - last conv layer + pool folded into precomputed pooled rows B/BS/BS2
   (g = (B h)(W0-W2) + (BS h)W1 + (BS2 h)(2W2) + cnt*b2): removes 2 spmv
   passes and 2 AllGathers.
 - split AllGather: table rows are relabeled so each rank's local rows
   [0,NPIECE0) form a contiguous leading block of the table; the AG for
   that block fires mid-pass (hidden under remaining gather descriptor
   generation) and only the small tail AG is exposed.
 - incremental table writes: transpose/scale/DMA of table chunks runs
   every 2 windows inside the pass instead of as a post-pass phase.
 - pool phase folded into the last pass the same way (per-chunk
   transpose+pool matmul as H windows complete).
"""
import numpy as np
import concourse.bacc as bacc
import concourse.bass as bass
import concourse.mybir as mybir
import concourse.tile as tile
from concourse.masks import make_identity

F32 = mybir.dt.float32
BF16 = mybir.dt.bfloat16
I16 = mybir.dt.int16
AF = mybir.ActivationFunctionType
ALU = mybir.AluOpType

P = 128
CH = 128
NUM_HIDDENS = 3
NUM_GRAPHS = 64
D_OUT_HID = 32
D_OUT = 16
NCORES = 8

NL = 6272            # padded nodes per core (98 subwindows of 64)
NTOT = NL * NCORES   # 50176 padded total
SW = 64              # subwindow width (dst cols per scatter matmul)
SPW = 7              # subwindows per PSUM window
WW = SW * SPW        # 448 dst per window
NSUB = NL // SW      # 98
NW = NSUB // SPW     # 14 windows
NCHN = NL // P       # 49 node-major chunks per core
BASE_HI = NTOT - 32768  # 17408; lo rows [0,32768), hi rows [17408, NTOT)

N_CONV = 1 + NUM_HIDDENS      # device conv layers (last conv folded into pool)
# split-AG: table rows are rank-major within two pieces; piece 0 holds each
# rank's local rows [0, NPIECE0) (= windows 0..9), piece 1 the rest.
NPIECE0 = NL                  # single-piece AG (split-AG regressed: Local
NPIECE1 = 0                   # collective output runs at ~half Shared BW)
T0TOT = NPIECE0 * NCORES
MID_W = None                  # no mid-pass AG

import os as _os_mod
SP = bool(int(_os_mod.environ.get("CHEB_SP", "0")))  # dma_gather single_packet


def cdiv(a, b):
    return (a + b - 1) // b


class Prep:
    """Host-side: permutation, edge bucketing, idx streams, m matrices."""

    def __init__(self, n_nodes, edge_index):
        N = self.N = n_nodes
        src = np.asarray(edge_index[0], dtype=np.int64)
        dst = np.asarray(edge_index[1], dtype=np.int64)
        keep = src != dst
        src, dst = src[keep], dst[keep]
        self.src, self.dst = src, dst
        deg = np.bincount(src, minlength=N).astype(np.float64)
        self.dinv = np.where(deg > 0, 1.0 / np.sqrt(np.maximum(deg, 1.0)),
                             0.0).astype(np.float32)

        # ---- serpentine in-degree balancing over 784 buckets of 64 slots
        in_deg = np.zeros(NTOT, np.int64)
        in_deg[:N] = np.bincount(dst, minlength=N)
        order = np.argsort(-in_deg, kind="stable")  # pads (deg 0) at end
        NB = NCORES * NSUB  # 784 buckets
        pos = np.empty(NTOT, np.int64)
        for r in range(SW):
            sl = order[r * NB:(r + 1) * NB]
            b = np.arange(NB) if r % 2 == 0 else np.arange(NB)[::-1]
            pos[sl] = (b % NCORES) * NL + (b // NCORES) * SW + r
        self.pos = pos  # old id -> new id

        # table-row relabeling: position (core k, local row r) -> table row
        # k*NPIECE0 + r for r < NPIECE0, else T0TOT + k*NPIECE1 + (r-NPIECE0).
        # Makes each AG piece a contiguous table block while keeping the
        # dst-side (window/subwindow) layout untouched.
        pidx = np.arange(NTOT)
        rr, kk = pidx % NL, pidx // NL
        self.trow = np.where(rr < NPIECE0, kk * NPIECE0 + rr,
                             T0TOT + kk * NPIECE1 + (rr - NPIECE0))

        ps = self.trow[pos[src]]
        pd = pos[dst]
        core = pd // NL
        rem = pd % NL
        sub = rem // SW
        dcol = rem % SW
        wdst = -self.dinv[dst]  # m value

        # ---- half split: fixed lo (<BASE_HI), fixed hi (>=32768),
        # flexible in [BASE_HI, 32768) balanced per (core, sub) bucket
        half = np.where(ps < BASE_HI, 0, np.where(ps >= 32768, 1, -1))
        bucket_id = core * NSUB + sub
        bord = np.argsort(bucket_id, kind="stable")
        bounds = np.searchsorted(bucket_id[bord], np.arange(NB + 1))
        for b in range(NB):
            seg = bord[bounds[b]:bounds[b + 1]]
            if len(seg) == 0:
                continue
            hm = half[seg]
            nlo = int((hm == 0).sum())
            flex = seg[hm == -1]
            tot = len(seg)
            want_lo = max(0, min(len(flex), (tot + 1) // 2 - nlo))
            half[flex[:want_lo]] = 0
            half[flex[want_lo:]] = 1

        w = sub // SPW
        s_in_w = sub % SPW
        cnt = np.zeros((NCORES, NW, SPW, 2), np.int64)
        np.add.at(cnt, (core, w, s_in_w, half), 1)
        K = cdiv(cnt.max(axis=0), 128)  # [NW, SPW, 2]
        K[:, :, 0] = np.maximum(K[:, :, 0], 1)
        self.K = K
        self.S = K.sum(axis=1) * 128 // 16  # [NW, 2] idx cols per call
        self.Stot = self.S.sum(axis=0)      # [2]
        self.nslot = int(K.sum())
        self.Kmax = int(K.sum(axis=1).max())

        # slot order: w asc, s asc, h asc, c asc (must match device loops)
        slot_of = np.zeros((NW, SPW, 2), np.int64)
        t = 0
        for wi in range(NW):
            for si in range(SPW):
                for hi in range(2):
                    slot_of[wi, si, hi] = t
                    t += int(K[wi, si, hi])

        self.idx_in = []
        self.m_in = []
        for k in range(NCORES):
            msk = core == k
            kh, kw, kss, kd, kps, kwd = (half[msk], w[msk], s_in_w[msk],
                                         dcol[msk], ps[msk], wdst[msk])
            idx_h = [np.zeros((P, max(int(self.Stot[h]), 1)), np.int16)
                     for h in range(2)]
            m_all = np.zeros((P, self.nslot * SW), np.float32)
            soff = [0, 0]
            for wi in range(NW):
                for h in range(2):
                    stream = []
                    for si in range(SPW):
                        emsk = (kw == wi) & (kss == si) & (kh == h)
                        e_d = kd[emsk]
                        e_ps = kps[emsk]
                        e_wd = kwd[emsk]
                        kk = int(K[wi, si, h])
                        npad = kk * 128 - len(e_ps)
                        assert npad >= 0, (wi, si, h, len(e_ps))
                        base = 0 if h == 0 else BASE_HI
                        stream.append(np.concatenate(
                            [e_ps - base, np.zeros(npad, np.int64)]))
                        sl0 = slot_of[wi, si, h]
                        i = np.arange(len(e_d))
                        np.add.at(m_all, (i % 128,
                                          (sl0 + i // 128) * SW + e_d), e_wd)
                    st = np.concatenate(stream)
                    S_w = int(self.S[wi, h])
                    if S_w:
                        wrapped = st.reshape(S_w, 16).T.astype(np.int16)
                        idx_h[h][:, soff[h]:soff[h] + S_w] = np.tile(
                            wrapped, (8, 1))
                    soff[h] += S_w
            self.idx_in.append(idx_h)
            self.m_in.append(m_all)


def build_kernel(prep: Prep):
    NW_MATS = 3 + NUM_HIDDENS * 3 + 3  # 12 conv + 3 pool-combo
    K, S = prep.K, prep.S
    nc = bacc.Bacc("TRN2", target_bir_lowering=False, debug=False,
                   num_devices=NCORES, num_swdge_queues=4)
    rg = [list(range(NCORES))]

    tbl0_d = nc.dram_tensor("tbl0", [NTOT, CH], BF16, kind="ExternalInput")
    h0_d = nc.dram_tensor("h0", [P, NL], BF16, kind="ExternalInput")
    idx_lo_d = nc.dram_tensor("idx_lo", [P, max(prep.Stot[0], 1)], I16,
                              kind="ExternalInput")
    idx_hi_d = nc.dram_tensor("idx_hi", [P, max(prep.Stot[1], 1)], I16,
                              kind="ExternalInput")
    m_d = nc.dram_tensor("m_all", [P, prep.nslot * SW], BF16,
                         kind="ExternalInput")
    dnm_d = nc.dram_tensor("dinv_nm", [P, NCHN], F32, kind="ExternalInput")
    d2nm_d = nc.dram_tensor("dinv2_nm", [P, NCHN], F32, kind="ExternalInput")
    pool_d = nc.dram_tensor("poolmat", [P, NCHN * 3 * NUM_GRAPHS], BF16,
                            kind="ExternalInput")
    w_d = nc.dram_tensor("Wcat", [P, NW_MATS * CH], BF16,
                         kind="ExternalInput")
    b_d = nc.dram_tensor("bcat", [P, 1 + NUM_HIDDENS], F32,
                         kind="ExternalInput")
    bias_d = nc.dram_tensor("bias_mat", [P, NUM_GRAPHS], F32,
                            kind="ExternalInput")
    p1_d = nc.dram_tensor("P1", [P, D_OUT_HID], F32, kind="ExternalInput")
    p2_d = nc.dram_tensor("P2", [D_OUT_HID, D_OUT], F32, kind="ExternalInput")
    pb1_d = nc.dram_tensor("pb1", [D_OUT_HID, 1], F32, kind="ExternalInput")
    pb2_d = nc.dram_tensor("pb2", [D_OUT, 1], F32, kind="ExternalInput")
    out_d = nc.dram_tensor("out", [D_OUT, NUM_GRAPHS], F32,
                           kind="ExternalOutput")

    with tile.TileContext(nc) as tc:
        with (
            tc.tile_pool(name="static", bufs=1) as st,
            tc.tile_pool(name="feat", bufs=1) as feat,
            tc.tile_pool(name="vlo", bufs=4) as vlo_p,
            tc.tile_pool(name="vhi", bufs=4) as vhi_p,
            tc.tile_pool(name="nm", bufs=2) as nm_p,
            tc.tile_pool(name="psu", bufs=2, space="PSUM") as psu,
            tc.tile_pool(name="psw", bufs=2, space="PSUM") as psw,
            tc.tile_pool(name="pst", bufs=2, space="PSUM") as pst,
            tc.tile_pool(name="psg", bufs=1, space="PSUM") as psg,
            tc.tile_pool(name="dram", bufs=1, space="DRAM") as dram,
        ):
            idx_lo = st.tile([P, max(prep.Stot[0], 1)], I16)
            nc.sync.dma_start(out=idx_lo[:], in_=idx_lo_d.ap())
            idx_hi = st.tile([P, max(prep.Stot[1], 1)], I16)
            nc.sync.dma_start(out=idx_hi[:], in_=idx_hi_d.ap())
            m_all = st.tile([P, prep.nslot * SW], BF16)
            nc.sync.dma_start(out=m_all[:], in_=m_d.ap())
            dnm = st.tile([P, NCHN], F32)
            nc.sync.dma_start(out=dnm[:], in_=dnm_d.ap())
            d2nm = st.tile([P, NCHN], F32)
            nc.sync.dma_start(out=d2nm[:], in_=d2nm_d.ap())
            poolm = st.tile([P, NCHN * 3 * NUM_GRAPHS], BF16)
            nc.sync.dma_start(out=poolm[:], in_=pool_d.ap())
            wcat = st.tile([P, NW_MATS * CH], BF16)
            nc.sync.dma_start(out=wcat[:], in_=w_d.ap())
            bcat = st.tile([P, 1 + NUM_HIDDENS], F32)
            nc.sync.dma_start(out=bcat[:], in_=b_d.ap())
            bias_t = st.tile([P, NUM_GRAPHS], F32)
            nc.sync.dma_start(out=bias_t[:], in_=bias_d.ap())
            p1_t = st.tile([P, D_OUT_HID], F32)
            nc.sync.dma_start(out=p1_t[:], in_=p1_d.ap())
            p2_t = st.tile([D_OUT_HID, D_OUT], F32)
            nc.sync.dma_start(out=p2_t[:], in_=p2_d.ap())
            pb1_t = st.tile([D_OUT_HID, 1], F32)
            nc.sync.dma_start(out=pb1_t[:], in_=pb1_d.ap())
            pb2_t = st.tile([D_OUT, 1], F32)
            nc.sync.dma_start(out=pb2_t[:], in_=pb2_d.ap())

            H = feat.tile([P, NL], BF16, name="H")
            nc.sync.dma_start(out=H[:], in_=h0_d.ap())
            T1 = feat.tile([P, NL], BF16, name="T1")

            # warmup collectives: the first AllGather/AllReduce pay ~15-20us
            # of ring/ENCD cold-start; fire tiny dummies here so that cost
            # overlaps pass-1's gather generation instead of boundary 1.
            # ident emitted here (first use is chunk_writer at ~45us) so
            # the first gather calls lead the GpSimd queue
            ident = st.tile([P, P], BF16)
            make_identity(nc, ident[:])
            wu_in = dram.tile([NCORES, CH], BF16, name="wu_in")
            wu_out = dram.tile([NCORES * NCORES, CH], BF16, name="wu_out",
                               addr_space="Shared")
            nc.gpsimd.collective_compute(
                "AllGather", ALU.bypass, replica_groups=rg,
                ins=[wu_in[:].opt()], outs=[wu_out[:].opt()])
            wr_in = dram.tile([P, 1], F32, name="wr_in")
            wr_out = dram.tile([P, 1], F32, name="wr_out",
                               addr_space="Shared")
            nc.gpsimd.collective_compute(
                "AllReduce", ALU.add, replica_groups=rg,
                ins=[wr_in[:].opt()], outs=[wr_out[:].opt()])

            def spmv(table_ap, evict, chunk_writer=None, mid_ag=None):
                """One spmv pass over all windows; evict(w, psum_tile).

                chunk_writer(j0, j1) is invoked after every odd window to
                flush completed table chunks; mid_ag() is emitted before
                window MID_W's gathers so the piece-0 AllGather flies while
                the remaining windows' descriptors are generated.
                """
                soff = [0, 0]
                slot = 0
                for w in range(NW):
                    if mid_ag is not None and w == MID_W:
                        mid_ag()
                    vts = []
                    for h, (idx_t, v_pool) in enumerate(
                        ((idx_lo, vlo_p), (idx_hi, vhi_p))
                    ):
                        ck = int(K[w, :, h].sum())
                        if ck == 0:
                            vts.append(None)
                            continue
                        base = 0 if h == 0 else BASE_HI
                        # split the last window's calls so the tail's
                        # drain->matmul chain is gated by a half-size call
                        if w == NW - 1 and ck >= 2:
                            c0 = (ck + 1) // 2
                            parts = [(0, c0), (c0, ck - c0)]
                        else:
                            parts = [(0, ck)]
                        segs = []
                        for b0, nb in parts:
                            v = v_pool.tile([P, prep.Kmax * CH], BF16,
                                            tag=f"v{h}")
                            nc.gpsimd.dma_gather(
                                out_ap=v[:, :nb * CH].rearrange(
                                    "p (c e) -> p c e", e=CH),
                                in_ap=table_ap[base:base + 32768, :],
                                idxs_ap=idx_t[:, soff[h] + b0 * 8:
                                              soff[h] + (b0 + nb) * 8],
                                num_idxs=nb * 128,
                                num_idxs_reg=nb * 128,
                                elem_size=CH,
                                single_packet=SP,
                                queue_num=1 + ((2 * w + h) % 3),
                            )
                            segs.append((v, nb))
                        vts.append(segs)
                        soff[h] += int(S[w, h])
                    u = psu.tile([P, WW], F32, tag="u")
                    for s in range(SPW):
                        ntot_s = int(K[w, s, 0] + K[w, s, 1])
                        i = 0
                        for h in range(2):
                            coff = int(K[w, :s, h].sum())
                            for c in range(int(K[w, s, h])):
                                b = coff + c
                                vt, lb = vts[h][0][0], b
                                if b >= vts[h][0][1]:
                                    vt = vts[h][1][0]
                                    lb = b - vts[h][0][1]
                                nc.tensor.matmul(
                                    out=u[:, s * SW:(s + 1) * SW],
                                    lhsT=vt[:, lb * CH:(lb + 1) * CH],
                                    rhs=m_all[:, slot * SW:(slot + 1) * SW],
                                    start=(i == 0),
                                    stop=(i == ntot_s - 1),
                                )
                                slot += 1
                                i += 1
                    evict(w, u)
                    if chunk_writer is not None and w % 2 == 1:
                        chunk_writer(7 * (w // 2), 7 * (w // 2) + 7)

            def write_chunks(src_cm, scale_nm, agi, j0, j1):
                """nm-transpose src_cm chunks [j0,j1) (scaled) -> agi.

                All chunks of the burst are scaled into one wide SBUF tile
                and written with a single DMA so the per-DMA fixed cost is
                amortized (one 32KB DMA per chunk serialized the pipeline
                at ~1.65us/chunk).
                """
                nb = j1 - j0
                ow = nm_p.tile([P, 7 * P], BF16, tag="ow")
                for c, j in enumerate(range(j0, j1)):
                    pt = pst.tile([P, P], BF16, tag="pt")
                    nc.tensor.transpose(pt[:], src_cm[:, j * P:(j + 1) * P],
                                        ident[:])
                    nc.vector.tensor_scalar(
                        out=ow[:, c * P:(c + 1) * P], in0=pt[:],
                        scalar1=scale_nm[:, j:j + 1],
                        scalar2=None, op0=ALU.mult)
                nc.sync.dma_start(
                    out=agi[j0 * P:j1 * P, :].rearrange(
                        "(c p) e -> p c e", p=P),
                    in_=ow[:, :nb * P].rearrange("p (c e) -> p c e", e=CH))

            def ag_full(agi, tbl):
                nc.gpsimd.collective_compute(
                    "AllGather", ALU.bypass, replica_groups=rg,
                    ins=[agi[:].opt()], outs=[tbl[:].opt()])

            n_layers = N_CONV
            tbl_cur = tbl0_d.ap()
            pg = psg.tile([P, 3 * NUM_GRAPHS], F32, name="pool")
            for l in range(n_layers):
                wofs = (0 if l == 0 else 3 + 3 * (l - 1)) * CH

                def ev1(w, u, l=l):
                    sl = slice(w * WW, (w + 1) * WW)
                    nc.vector.tensor_copy(out=T1[:, sl], in_=u[:, :WW])

                tblB = dram.tile([NTOT, CH], BF16, tag="tblB",
                                 name=f"tblB_{l}", addr_space="Shared")
                agiB = dram.tile([NL, CH], BF16, tag="agi", name=f"agiB_{l}")

                def cwB(j0, j1, agiB=agiB):
                    write_chunks(T1, d2nm, agiB, j0, j1)

                spmv(tbl_cur, ev1, chunk_writer=cwB)
                ag_full(agiB, tblB)

                def ev2(w, u, l=l, wofs=wofs):
                    sl = slice(w * WW, (w + 1) * WW)
                    t2 = nm_p.tile([P, WW], BF16, tag="t2")
                    nc.vector.tensor_tensor(out=t2[:], in0=u[:, :WW],
                                            in1=H[:, sl], op=ALU.subtract)
                    pw = psw.tile([P, WW], F32, tag="pw")
                    nc.tensor.matmul(out=pw[:], lhsT=wcat[:, wofs:wofs + CH],
                                     rhs=H[:, sl], start=True, stop=False)
                    nc.tensor.matmul(out=pw[:],
                                     lhsT=wcat[:, wofs + CH:wofs + 2 * CH],
                                     rhs=T1[:, sl], start=False, stop=False)
                    nc.tensor.matmul(out=pw[:],
                                     lhsT=wcat[:, wofs + 2 * CH:wofs + 3 * CH],
                                     rhs=t2[:], start=False, stop=True)
                    if l >= 1:
                        nc.scalar.activation(out=H[:, sl], in_=pw[:],
                                             func=AF.Silu,
                                             bias=bcat[:, l:l + 1], scale=1.0)
                    else:
                        nc.vector.tensor_scalar(
                            out=H[:, sl], in0=pw[:],
                            scalar1=bcat[:, l:l + 1], scalar2=None,
                            op0=ALU.add)

                if l < n_layers - 1:
                    tblA = dram.tile([NTOT, CH], BF16, tag="tblA",
                                     name=f"tblA_{l}", addr_space="Shared")
                    agiA = dram.tile([NL, CH], BF16, tag="agi2",
                                     name=f"agiA_{l}")

                    def cwA(j0, j1, agiA=agiA):
                        write_chunks(H, dnm, agiA, j0, j1)

                    spmv(tblB[:], ev2, chunk_writer=cwA)
                    ag_full(agiA, tblA)
                    tbl_cur = tblA[:]
                else:
                    # last conv pass: fold pooling in as the chunk writer
                    def pool_chunks(j0, j1):
                        for j in range(j0, j1):
                            pt = pst.tile([P, P], BF16, tag="pt")
                            nc.tensor.transpose(
                                pt[:], H[:, j * P:(j + 1) * P], ident[:])
                            hn = nm_p.tile([P, P], BF16, tag="o")
                            nc.vector.tensor_copy(out=hn[:], in_=pt[:])
                            nc.tensor.matmul(
                                out=pg[:], lhsT=hn[:],
                                rhs=poolm[:, j * 3 * NUM_GRAPHS:
                                          (j + 1) * 3 * NUM_GRAPHS],
                                start=(j == 0), stop=(j == NCHN - 1))

                    spmv(tblB[:], ev2, chunk_writer=pool_chunks)

            gsb = nm_p.tile([P, 3 * NUM_GRAPHS], BF16, tag="t2", name="gsb")
            nc.vector.tensor_copy(out=gsb[:], in_=pg[:])
            # combine: U = C0^T g0 + C1^T g1 + C2^T g2
            up = psw.tile([P, NUM_GRAPHS], F32, tag="pw", name="ucomb")
            for t in range(3):
                nc.tensor.matmul(
                    out=up[:],
                    lhsT=wcat[:, (12 + t) * CH:(13 + t) * CH],
                    rhs=gsb[:, t * NUM_GRAPHS:(t + 1) * NUM_GRAPHS],
                    start=(t == 0), stop=(t == 2))
            usb = nm_p.tile([P, NUM_GRAPHS], F32, tag="o", name="usb")
            nc.vector.tensor_copy(out=usb[:], in_=up[:])
            ar_in = dram.tile([P, NUM_GRAPHS], F32, tag="ar_in", name="ar_in")
            ar_out = dram.tile([P, NUM_GRAPHS], F32, tag="ar_out",
                               name="ar_out", addr_space="Shared")
            nc.sync.dma_start(out=ar_in[:], in_=usb[:])
            nc.gpsimd.collective_compute(
                "AllReduce", ALU.add, replica_groups=rg,
                ins=[ar_in[:].opt()], outs=[ar_out[:].opt()])
            gar = nm_p.tile([P, NUM_GRAPHS], F32, tag="t2", name="gar")
            nc.sync.dma_start(out=gar[:], in_=ar_out[:])
            gfull = nm_p.tile([P, NUM_GRAPHS], F32, tag="o", name="gfull")
            nc.vector.tensor_tensor(out=gfull[:], in0=gar[:], in1=bias_t[:],
                                    op=ALU.add)

            # ---- MLP (computed transposed)
            t1p = psg.tile([D_OUT_HID, NUM_GRAPHS], F32, tag="mp", name="m1")
            nc.tensor.matmul(out=t1p[:], lhsT=p1_t[:], rhs=gfull[:],
                             start=True, stop=True)
            s1 = nm_p.tile([D_OUT_HID, NUM_GRAPHS], F32, tag="t2", name="ms")
            nc.scalar.activation(out=s1[:], in_=t1p[:], func=AF.Relu,
                                 bias=pb1_t[:, 0:1], scale=1.0)
            t2p = psg.tile([D_OUT, NUM_GRAPHS], F32, tag="mp", name="m2")
            nc.tensor.matmul(out=t2p[:], lhsT=p2_t[:], rhs=s1[:],
                             start=True, stop=True)
            o2 = nm_p.tile([D_OUT, NUM_GRAPHS], F32, tag="o", name="mo")
            nc.vector.tensor_scalar(out=o2[:], in0=t2p[:],
                                    scalar1=pb2_t[:, 0:1], scalar2=None,
                                    op0=ALU.add)
            nc.sync.dma_start(out=out_d.ap(), in_=o2[:])

    nc.compile()
    return nc


def make_inputs(prep: Prep, x, W1, b1, Wh, bh, W2, b2, P1, pb1, P2, pb2,
                batch):
    import scipy.sparse as sp

    N = prep.N
    dinv = prep.dinv
    np_bf = mybir.dt.np(BF16)

    x = np.asarray(x, np.float32)
    batch = np.asarray(batch, np.int64)
    W2 = np.asarray(W2, np.float32)
    b2 = np.asarray(b2, np.float32)
    # permuted full arrays (padded)
    xp = np.zeros((NTOT, CH), np.float32)
    xp[prep.pos[:N]] = x
    dinv_p = np.zeros(NTOT, np.float32)
    dinv_p[prep.pos[:N]] = dinv
    batch_p = np.full(NTOT, -1, np.int64)
    batch_p[prep.pos[:N]] = batch

    # tbl0 is laid out in table-row order (trow relabeling)
    tbl0 = np.zeros((NTOT, CH), np.float32)
    tbl0[prep.trow] = xp * dinv_p[:, None]
    tbl0 = tbl0.astype(np_bf)

    # pooled rows: B (one-hot), BS = B@S, BS2 = BS@S  (original node ids)
    w_e = (-dinv[prep.src] * dinv[prep.dst]).astype(np.float64)
    Smat = sp.coo_matrix((w_e, (prep.dst, prep.src)), shape=(N, N)).tocsr()
    Bmat = sp.coo_matrix((np.ones(N), (batch, np.arange(N))),
                         shape=(NUM_GRAPHS, N)).tocsr()
    BS = np.asarray((Bmat @ Smat).todense())
    BS2 = BS @ Smat  # dense @ sparse -> dense [64, N]
    # permute+pad to [64, NTOT]
    Bp = np.zeros((NUM_GRAPHS, NTOT), np.float32)
    BSp = np.zeros((NUM_GRAPHS, NTOT), np.float32)
    BS2p = np.zeros((NUM_GRAPHS, NTOT), np.float32)
    Bp[batch_p[prep.pos[:N]], prep.pos[:N]] = 1.0
    BSp[:, prep.pos[:N]] = BS
    BS2p[:, prep.pos[:N]] = BS2

    cnt = np.bincount(batch, minlength=NUM_GRAPHS).astype(np.float32)
    bias_mat = np.outer(b2, cnt).astype(np.float32)  # [128, 64]

    # conv weights (12) + pool-combo (3): C0=W0-W2, C1=W1, C2=2*W2
    wcat = np.concatenate(
        [W1[k] for k in range(3)]
        + [Wh[i][k] for i in range(NUM_HIDDENS) for k in range(3)]
        + [W2[0] - W2[2], W2[1], 2.0 * W2[2]], axis=1).astype(np_bf)
    bcat = np.stack([b1] + [bh[i] for i in range(NUM_HIDDENS)],
                    axis=1).astype(np.float32)

    in_maps = []
    for k in range(NCORES):
        blk = slice(k * NL, (k + 1) * NL)
        d_blk = dinv_p[blk]
        h0 = np.ascontiguousarray(xp[blk].T).astype(np_bf)  # [128, NL]
        d_nm = d_blk.reshape(NCHN, P).T.astype(np.float32)
        d2_nm = (2.0 * d_blk).reshape(NCHN, P).T.astype(np.float32)
        # poolmat node-major: [128, NCHN*192]; chunk j cols: [B | BS | BS2]
        pm = np.zeros((P, NCHN * 3 * NUM_GRAPHS), np.float32)
        for j in range(NCHN):
            nsl = slice(k * NL + j * P, k * NL + (j + 1) * P)
            o = j * 3 * NUM_GRAPHS
            pm[:, o:o + NUM_GRAPHS] = Bp[:, nsl].T
            pm[:, o + NUM_GRAPHS:o + 2 * NUM_GRAPHS] = BSp[:, nsl].T
            pm[:, o + 2 * NUM_GRAPHS:o + 3 * NUM_GRAPHS] = BS2p[:, nsl].T
        in_maps.append(dict(
            tbl0=tbl0,
            h0=h0,
            idx_lo=prep.idx_in[k][0],
            idx_hi=prep.idx_in[k][1],
            m_all=prep.m_in[k].astype(np_bf),
            dinv_nm=np.ascontiguousarray(d_nm),
            dinv2_nm=np.ascontiguousarray(d2_nm),
            poolmat=pm.astype(np_bf),
            Wcat=np.ascontiguousarray(wcat),
            bcat=np.ascontiguousarray(bcat),
            bias_mat=bias_mat,
            P1=np.asarray(P1, np.float32),
            P2=np.asarray(P2, np.float32),
            pb1=np.asarray(pb1, np.float32)[:, None],
            pb2=np.asarray(pb2, np.float32)[:, None],
        ))
    return in_maps


_CACHE = {}


def kernel(x, edge_index, batch, W1, b1, Wh, bh, W2, b2, P1, pb1, P2, pb2):
    import os as _os
    from concourse import bass_utils as _bu

    x = np.asarray(x, dtype=np.float32)
    edge_index = np.asarray(edge_index)
    batch = np.asarray(batch)
    key = (x.shape, edge_index.shape)
    if _CACHE.get("key") != key:
        prep = Prep(x.shape[0], edge_index)
        nc = build_kernel(prep)
        _CACHE.update(key=key, prep=prep, nc=nc)
    prep, nc = _CACHE["prep"], _CACHE["nc"]
    in_maps = make_inputs(prep, x, W1, b1, Wh, bh, W2, b2, P1, pb1, P2, pb2,
                          batch)
    trace = bool(int(_os.environ.get("CHEB_TRACE", "0")))
    res = _bu.run_bass_kernel_spmd(nc, in_maps, core_ids=list(range(NCORES)),
                                   trace=trace)
    _CACHE["res"] = res
    if res.exec_time_ns is not None:
        print(f"HW exec time: {res.exec_time_ns} ns")
    out = res.results[0]["out"]  # [16, 64]
    return np.ascontiguousarray(out.T).astype(np.float32)
